# revision 5
# baseline (speedup 1.0000x reference)
"""Trainium2 Bass kernel for nn_MultiHeadAttention (B=4, S=2048, D=1024,
H=16, DK=DV=64) with key-padding + causal mask, exp-without-max softmax.

Sharding: 8 cores = (batch b = core//2) x (head half = core%2, 8 heads each).

v4 design (per core), all matmuls bf16 (fp32 PSUM accumulate):
 - QK ROW-TILED (64x128): per k-tile the two co-scheduled heads run as two
   K=64 matmuls on row-tiles (0,0)/(64,0), concurrent in the PE array ->
   ~N cycles for both heads instead of 2N.
 - PV COL-TILED (128x64): ctx^T for the two heads -> one [128,512] PSUM
   tile, col-tiles (0,0)/(0,64), concurrent -> ~N cycles per k-tile.
 - softmax denominators: E tiles accumulated per-group into esum (bf16 DVE
   adds at 2x rate; the group's first exp writes straight into esum and PV
   reads esum for that tile), then one col-tiled (128x32) ones-matmul pair
   per group; host divides.
 - ALL projection pieces (not just chunk 3) are woven into the attention
   groups as PE filler so the Scalar engine's exp stream never starves and
   the PE never idles waiting for exps: chunk j's k/v pieces run inside
   groups (j,0..3) before the diagonal tiles (seq is ascending, diag last),
   chunk j+1's q pieces inside (j,*).
"""

import sys

sys.path.insert(0, "/opt/trn_rl_repo")

import ml_dtypes
import numpy as np

import concourse.bass as bass
import concourse.mybir as mybir
import concourse.tile as tile
from concourse import bacc
from concourse.bass_utils import run_bass_kernel_spmd

F32 = mybir.dt.float32
BF16 = mybir.dt.bfloat16
EXP = mybir.ActivationFunctionType.Exp
COPY = mybir.ActivationFunctionType.Copy

BF16NP = ml_dtypes.bfloat16

B, S, D = 4, 2048, 1024
H, DK, DV = 16, 64, 64
HPC = 8  # heads per core
FPC = HPC * DK  # projected features per core (512)
NTT = S // 128  # 16 token tiles
NQC = S // 512  # 4 q-chunks
TC = 512  # projection token-chunk size
NDC = D // 128  # 8 contraction chunks
SCALE = 1.0 / np.sqrt(DK)
NEG = 480.0  # additive mask magnitude: exp(x - 480) == 0.0 in fp32


def build_nc():
    nc = bacc.Bacc()

    xd = {n: nc.dram_tensor(f"x{n}", [NQC, 128, NDC * TC], BF16, kind="ExternalInput") for n in "qkv"}
    wd = {n: nc.dram_tensor(f"w{n}", [128, NDC * FPC], BF16, kind="ExternalInput") for n in "qkv"}
    maskp_d = nc.dram_tensor("maskp", [128, NTT], F32, kind="ExternalInput")
    tri_d = nc.dram_tensor("trid", [128, 2 * 128], BF16, kind="ExternalInput")
    out_d = nc.dram_tensor("out", [NQC, 4, 128, 512], F32, kind="ExternalOutput")
    sums_d = nc.dram_tensor("sums", [2, NQC, 4, 512], F32, kind="ExternalOutput")

    with tile.TileContext(nc) as tc:
        with (
            tc.tile_pool(name="const", bufs=1) as cpool,
            tc.tile_pool(name="big", bufs=1) as big,
            tc.tile_pool(name="xp", bufs=6) as xpool,
            tc.tile_pool(name="e", bufs=4) as epool,
            tc.tile_pool(name="es", bufs=2) as espool,
            tc.tile_pool(name="ob", bufs=4) as obpool,
            tc.tile_pool(name="mm", bufs=2, space="PSUM") as psmm,
            tc.tile_pool(name="sc", bufs=2, space="PSUM") as pssc,
            tc.tile_pool(name="ctx", bufs=2, space="PSUM") as psctx,
        ):
            # ---------------- warmup (no DMA dependency)
            warm = cpool.tile([128, 512], BF16)
            nc.vector.memset(warm, 0.0)

            def warmup(n0, n1):
                for wi in range(n0, n1):
                    wps = psctx.tile([128, 512], F32, tag="ctx", name=f"warm{wi}")
                    nc.tensor.matmul(
                        wps, lhsT=warm[:, 0:128], rhs=warm, start=True, stop=True
                    )

            warmup(0, 7)

            # ---------------- persistent tiles
            kT_all = big.tile([128, 4, S], BF16)
            mv = big.tile([128, NTT, HPC, DV], BF16)  # [k128, ktile, h, dv]
            qt = [big.tile([128, 4, 512], BF16, name=f"qt{i}") for i in range(2)]
            ones = cpool.tile([128, 1], BF16)
            nc.vector.memset(ones, 1.0)

            w_sb = {}

            def load_w(n, nparts=2):
                w_sb[n] = big.tile([128, NDC, FPC], BF16, name=f"w{n}")
                step = NDC // nparts
                for pi in range(nparts):
                    nc.sync.dma_start(
                        out=w_sb[n][:, pi * step : (pi + 1) * step, :],
                        in_=bass.AP(
                            tensor=wd[n],
                            offset=pi * step * FPC,
                            ap=[[NDC * FPC, 128], [1, step * FPC]],
                        ),
                    )

            def load_x(name, cn, nparts=2):
                x = xpool.tile([128, NDC, TC], BF16, tag="x", name=f"x{name}{cn}")
                step = NDC // nparts
                for pi in range(nparts):
                    nc.sync.dma_start(
                        out=x[:, pi * step : (pi + 1) * step, :],
                        in_=bass.AP(
                            tensor=xd[name],
                            offset=cn * 128 * NDC * TC + pi * step * TC,
                            ap=[[NDC * TC, 128], [1, step * TC]],
                        ),
                    )
                return x

            xs = {}  # (name, chunk) -> x tile

            def project_part(name, cn, pieces=(0, 1, 2, 3)):
                """Run projection matmuls for one name over chunk cn.
                pieces selects v token-tiles or q/k feature-chunks."""
                x = xs[name, cn]
                qt_c = qt[cn % 2]
                if name == "v":
                    for tt in pieces:
                        t = cn * 4 + tt
                        ps = psmm.tile([128, FPC], F32, tag="mm")
                        for dc in range(NDC):
                            nc.tensor.matmul(
                                ps,
                                lhsT=x[:, dc, tt * 128 : (tt + 1) * 128],
                                rhs=w_sb[name][:, dc, :],
                                start=(dc == 0),
                                stop=(dc == NDC - 1),
                            )
                        # one strided copy interleaves all 8 heads
                        nc.vector.tensor_copy(
                            mv[:, t, :, :],
                            ps[:, :].rearrange("p (h d) -> p h d", h=HPC),
                        )
                else:
                    for fc in pieces:
                        ps = psmm.tile([128, TC], F32, tag="mm")
                        for dc in range(NDC):
                            nc.tensor.matmul(
                                ps,
                                lhsT=w_sb[name][:, dc, fc * 128 : (fc + 1) * 128],
                                rhs=x[:, dc, :],
                                start=(dc == 0),
                                stop=(dc == NDC - 1),
                            )
                        if name == "q":
                            nc.vector.tensor_copy(qt_c[:, fc, :], ps)
                        else:
                            nc.vector.tensor_copy(
                                kT_all[:, fc, cn * TC : (cn + 1) * TC], ps
                            )

            def attention_group(fc, j, fillers=()):
                """fillers: dict idx -> list of zero-arg closures emitted as PE
                filler after that kt index's QK+stage2."""
                qt_j = qt[j % 2]
                nkt = 4 * (j + 1)
                ctx = psctx.tile([128, 512], F32, tag="ctx", name=f"ctx{fc}_{j}")
                esum = espool.tile([128, 2, 512], BF16, tag="es", name=f"es{fc}_{j}")

                def stage2(kt, first, last):
                    # exp + causal-mask + esum + col-tiled PV for an
                    # already-emitted score tile
                    p, off, sc = pend[kt]
                    if first:
                        # group's first tile covers all 512 q cols (off==0):
                        # exp straight into esum; PV reads esum
                        E = esum
                        nc.scalar.activation(
                            E, sc, EXP, scale=float(SCALE), bias=padbias[:, kt : kt + 1]
                        )
                    else:
                        E = epool.tile(
                            [128, 2, 512], BF16, tag="e", name=f"e{fc}_{j}_{kt}"
                        )
                        nc.scalar.activation(
                            E[:, :, off:],
                            sc[:, :, off:],
                            EXP,
                            scale=float(SCALE),
                            bias=padbias[:, kt : kt + 1],
                        )
                    if p >= 0:
                        # kill the sub-diagonal triangle of the [128,128] block
                        nc.vector.tensor_mul(
                            E[:, :, off : off + 128], E[:, :, off : off + 128], tri
                        )
                    if not first:
                        nc.vector.tensor_add(
                            esum[:, :, off:], esum[:, :, off:], E[:, :, off:]
                        )
                    for hh in range(2):
                        nc.tensor.matmul(
                            ctx[64 * hh : 64 * (hh + 1), off:],
                            lhsT=mv[:, kt, 2 * fc + hh, :],
                            rhs=E[:, hh, off:],
                            start=first,
                            stop=last,
                            tile_position=(0, 64 * hh),
                        )
                    if last:
                        ob = obpool.tile([128, 512], F32, tag="ob", name=f"ob{fc}_{j}")
                        nc.vector.tensor_copy(ob, ctx)
                        nc.sync.dma_start(out=out_d[j, fc], in_=ob)
                        sp = psmm.tile([128, 512], F32, tag="mm", name=f"sp{fc}_{j}")
                        for hh in range(2):
                            nc.tensor.matmul(
                                sp[32 * hh : 32 * hh + 1, :],
                                lhsT=ones,
                                rhs=esum[:, hh, :],
                                start=True,
                                stop=True,
                                tile_position=(0, 32 * hh),
                            )
                        sb = obpool.tile([33, 512], F32, tag="sb", name=f"sb{fc}_{j}")
                        for hh in range(2):
                            nc.vector.tensor_copy(
                                sb[32 * hh : 32 * hh + 1, :],
                                sp[32 * hh : 32 * hh + 1, :],
                            )
                            nc.sync.dma_start(
                                out=sums_d[hh, j, fc],
                                in_=sb[32 * hh : 32 * hh + 1, :],
                            )

                # ascending seq: full tiles first, diagonal tiles last, so the
                # current chunk's k/v projection fillers can complete inside
                # this group before its diagonal tiles need them, and the
                # kernel tail ends on the shortest exps.
                pend = {}
                for idx in range(nkt):
                    kt = idx
                    p = kt - 4 * j
                    off = 128 * p if p >= 0 else 0
                    sc = pssc.tile([128, 2, 512], F32, tag="sc", name=f"sc{fc}_{j}_{kt}")
                    for hh in range(2):
                        nc.tensor.matmul(
                            sc[:, hh, off:],
                            lhsT=kT_all[64 * hh : 64 * (hh + 1), fc, kt * 128 : (kt + 1) * 128],
                            rhs=qt_j[64 * hh : 64 * (hh + 1), fc, off:],
                            start=True,
                            stop=True,
                            tile_position=(64 * hh, 0),
                        )
                    pend[kt] = (p, off, sc)
                    if idx > 0:
                        stage2(idx - 1, idx - 1 == 0, idx - 1 == nkt - 1)
                    for f in fillers.get(idx, ()):
                        f()
                stage2(nkt - 1, nkt == 1, True)

            # ---- software pipeline.
            # chunk 0: interleave x/w loads with projection emission so the
            # first matmul's operands are at the head of the DMA queues.
            xv0 = xpool.tile([128, NDC, TC], BF16, tag="x", name="xv0")
            xs["v", 0] = xv0
            for pi in range(4):
                nc.sync.dma_start(
                    out=w_sb.setdefault("v", big.tile([128, NDC, FPC], BF16, name="wv"))[
                        :, 2 * pi : 2 * (pi + 1), :
                    ],
                    in_=bass.AP(
                        tensor=wd["v"],
                        offset=pi * 2 * FPC,
                        ap=[[NDC * FPC, 128], [1, 2 * FPC]],
                    ),
                )
                nc.sync.dma_start(
                    out=xv0[:, 2 * pi : 2 * (pi + 1), :],
                    in_=bass.AP(
                        tensor=xd["v"],
                        offset=pi * 2 * TC,
                        ap=[[NDC * TC, 128], [1, 2 * TC]],
                    ),
                )
            # constants (needed from the first exp, ~25us in)
            tri = cpool.tile([128, 2, 128], BF16)
            nc.sync.dma_start(out=tri, in_=tri_d[:, :].rearrange("p (a b) -> p a b", a=2))
            maskcol = cpool.tile([128, NTT], F32)
            nc.sync.dma_start(out=maskcol, in_=maskp_d[:, :])
            padbias = cpool.tile([128, NTT], F32)
            # (m-1)*NEG: 0 for valid keys, -NEG for padded
            nc.scalar.activation(padbias, maskcol, COPY, scale=float(NEG), bias=-float(NEG))
            project_part("v", 0)
            for n in "qk":
                xs[n, 0] = load_x(n, 0, nparts=4)
                load_w(n)
                project_part(n, 0)

            # filler placement: chunk j's k piece fc and v pieces run inside
            # groups (j, *) before their diagonal tiles; chunk j+1's q pieces
            # run inside groups (j, *).  One piece ~= 8 matmuls ~= 1.8us of
            # PE filler under the group's exp stream.
            def P(name, cn, piece):
                return lambda: project_part(name, cn, (piece,))

            fill = {
                (0, 0): {1: [P("q", 1, 0)]},
                (0, 1): {1: [P("q", 1, 1)]},
                (0, 2): {1: [P("q", 1, 2)]},
                (0, 3): {1: [P("q", 1, 3)]},
                (1, 0): {0: [P("k", 1, 0)], 1: [P("v", 1, 0)], 2: [P("v", 1, 1)],
                         3: [P("v", 1, 2)], 4: [P("v", 1, 3)]},
                (1, 1): {0: [P("k", 1, 1)], 2: [P("q", 2, 0)]},
                (1, 2): {0: [P("k", 1, 2)], 2: [P("q", 2, 1)]},
                (1, 3): {0: [P("k", 1, 3)], 2: [P("q", 2, 2)]},
                (2, 0): {0: [P("k", 2, 0)], 1: [P("v", 2, 0)], 2: [P("v", 2, 1)],
                         3: [P("v", 2, 2)], 4: [P("v", 2, 3)], 6: [P("q", 2, 3)]},
                (2, 1): {0: [P("k", 2, 1)], 2: [P("q", 3, 0)]},
                (2, 2): {0: [P("k", 2, 2)], 2: [P("q", 3, 1)]},
                (2, 3): {0: [P("k", 2, 3)], 2: [P("q", 3, 2)]},
                (3, 0): {0: [P("k", 3, 0)], 1: [P("v", 3, 0)], 2: [P("v", 3, 1)],
                         3: [P("v", 3, 2)], 4: [P("v", 3, 3)]},
                (3, 1): {0: [P("k", 3, 1)], 2: [P("q", 3, 3)]},
                (3, 2): {0: [P("k", 3, 2)]},
                (3, 3): {0: [P("k", 3, 3)]},
            }

            for j in range(NQC):
                for fc in range(4):
                    if j < 3 and fc == 0:
                        xs["q", j + 1] = load_x("q", j + 1)
                    elif j < 3 and fc == 1:
                        xs["k", j + 1] = load_x("k", j + 1)
                    elif j < 3 and fc == 2:
                        xs["v", j + 1] = load_x("v", j + 1)
                    attention_group(fc, j, fill.get((j, fc), {}))
    nc.finalize()
    return nc


_NC_CACHE = {}


def _get_nc():
    if "nc" not in _NC_CACHE:
        _NC_CACHE["nc"] = build_nc()
    return _NC_CACHE["nc"]


def _pack_x(X):
    # [S, D] fp32 -> [chunk, p, dc*TC+col] bf16 (partition-major per chunk)
    A = np.ascontiguousarray(X.T).reshape(NDC, 128, NQC, TC)
    return np.ascontiguousarray(A.transpose(2, 1, 0, 3).reshape(NQC, 128, NDC * TC)).astype(BF16NP)


def _pack_w(Wslice):
    # [FPC, D] fp32 -> [p, dc*FPC+f] bf16
    A = np.ascontiguousarray(Wslice.T).reshape(NDC, 128, FPC)
    return np.ascontiguousarray(A.transpose(1, 0, 2).reshape(128, NDC * FPC)).astype(BF16NP)


def _host_consts():
    kk = np.arange(128)[:, None]
    qc = np.arange(128)[None, :]
    tri1 = (qc >= kk).astype(np.float32)  # upper triangular incl diagonal
    tri = np.concatenate([tri1, tri1], axis=1).astype(BF16NP)  # both heads
    return np.ascontiguousarray(tri)


def kernel(Q, K, V, mask, W_Q, W_K, W_V, b_Q, b_K, b_V, _run=None):
    Q, K, V = (np.asarray(a, np.float32) for a in (Q, K, V))
    W_Q, W_K, W_V = (np.asarray(a, np.float32) for a in (W_Q, W_K, W_V))
    b_Q, b_K, b_V = (np.asarray(a, np.float32) for a in (b_Q, b_K, b_V))
    mask = np.asarray(mask)

    # biases are folded on host into nothing (this problem ships zeros);
    # nonzero biases would need a device-side add, guard against that.
    assert not b_Q.any() and not b_K.any() and not b_V.any(), "nonzero biases unsupported"

    nc = _get_nc()
    tri = _host_consts()

    in_maps = []
    for c in range(8):
        b, half = c // 2, c % 2
        fsl = slice(half * FPC, (half + 1) * FPC)
        m = {
            "xq": _pack_x(Q[b]),
            "xk": _pack_x(K[b]),
            "xv": _pack_x(V[b]),
            "wq": _pack_w(W_Q[fsl]),
            "wk": _pack_w(W_K[fsl]),
            "wv": _pack_w(W_V[fsl]),
            "maskp": np.ascontiguousarray(mask[b].reshape(NTT, 128).T).astype(np.float32),
            "trid": tri.reshape(128, 256),
        }
        in_maps.append(m)

    run = _run or (lambda n, im: run_bass_kernel_spmd(n, im, core_ids=list(range(8))))
    res = run(nc, in_maps)

    out = np.empty((B, S, H * DV), np.float32)
    for c in range(8):
        b, half = c // 2, c % 2
        r = np.asarray(res.results[c]["out"], np.float32).reshape(NQC, 4, 2, DV, 512)
        s = np.asarray(res.results[c]["sums"], np.float32)  # [2, NQC, 4, 512]
        den = s.transpose(1, 2, 0, 3) + 1e-8  # [j, fc, hh, q]
        ctx = r / den[:, :, :, None, :]  # [j, fc, hh, dv, q]
        out[b, :, half * FPC : (half + 1) * FPC] = ctx.transpose(0, 4, 1, 2, 3).reshape(S, FPC)
    return out
